# revision 11
# baseline (speedup 1.0000x reference)
"""Trainium2 Bass kernel for nn_DenoisingSharpening.

Contract: kernel(**inputs) takes the FULL unsharded inputs
(images [8,64,64,64,3] f32, params [8,64,7] f32, k [] f32) and returns
the FULL output [8,64,64,64,3] f32.

Strategy
--------
Embarrassingly data-parallel over the N = B*P = 512 images; 64 images per
NeuronCore, one half-image (32 rows) per SBUF partition -> 128 partitions.

Host side: reflect-pads each image to [34, 66, 3] halves and precomputes all
per-image scalars (gaussian kernels, bilateral scales, thresholds) so the
device only streams elementwise math.

Device side (per core, Tile framework):
  * bilateral 3x3: only 4 unique tap fields (W, NW, N, NE) are computed;
    the mirror taps are shifted reads of the same fields (the color kernel
    is symmetric between pixel pairs).
  * neighbor diffs are computed in f32 (exact), then converted to f16 on the
    GPSIMD engine; products/accumulations run as f16 tensor_tensor (2x DVE).
  * ck3 = exp(-d2 + log(ws)) is emitted channel-broadcast in f16 directly by
    the ACT engine (free scale/bias + broadcast read).
  * bf = x + sum(ck*diff)/sum(ck) (algebraically identical to the reference).
  * the separable gaussian detail is reconstructed from the SAME diff fields
    (diff-space formulation) so f16 stays precision-safe:
        H = dW + dW(+E), V = dN + dN(+S), G = H + V,
        s2 = 2H - H(up) - H(down),
        detail = -bE*(G - bE*s2) = -bE*inner
    with the -bE folded into abs-scale and the lambda scalar.
  * detail-mask via tanh (same ACT table set as exp/square/abs -> one
    table load), noise chain fused via tensor_scalar / STT.
  * skip-means via free accum_out reductions + stream_shuffle partition
    pair exchange.
"""

import numpy as np

N_CORES = 8
B, PP, H, W, C = 8, 64, 64, 64, 3
NIMG = B * PP  # 512
HALVES = 2 * NIMG  # 1024 half-images, 128 per core
PR, PC = 34, 66  # padded half-image rows/cols
ROWS_PER_HALF = 32
CHUNKS = 4
CR = ROWS_PER_HALF // CHUNKS  # interior rows per chunk (8)
SLAB_R = CR + 2  # slab rows incl. halo (10)

NOISE_THRESH = 0.002
SKIP_THRESH = 1e-4
MEAN_N = float(C * H * W)  # 12288 elements per image mean

# params columns
(P_S, P_LOGE, P_LOGC, P_WSC, P_BE, P_KT, P_KTB, P_IGT, P_OFFGT, P_CLIP,
 P_SQL, P_NSQL, P_TN, P_NBE, P_PAD1, P_PAD2) = range(16)
NPARAM = 16

_CACHE = {}


# --------------------------------------------------------------------------
# host-side preprocessing
# --------------------------------------------------------------------------

def _host_prep(images, params, k):
    x = np.ascontiguousarray(images, dtype=np.float32).reshape(NIMG, H, W, C)
    xp = np.pad(x, ((0, 0), (1, 1), (1, 1), (0, 0)), mode="reflect")
    # halves: rows 0..33 and 32..65 of the padded [66,66,3]
    halves = np.stack([xp[:, 0:PR], xp[:, ROWS_PER_HALF:ROWS_PER_HALF + PR]], axis=1)
    halves = np.ascontiguousarray(halves, dtype=np.float32).reshape(HALVES, PR, PC, C)

    p = np.asarray(params, dtype=np.float32).reshape(NIMG, 7)
    sigma_s = np.clip(p[:, 0], 0.2, 5.0)
    sigma_r = np.clip(p[:, 1], 0.01, 1.0)
    sigma_f = np.clip(p[:, 2], 0.2, 3.0)
    lam = np.clip(p[:, 3], 0.1, 2.0)
    tau = np.clip(p[:, 4], 0.5, 5.0)
    gain = np.clip(p[:, 5], 0.2, 2.0)
    offset = np.clip(p[:, 6], 0.01, 1.0)

    def gauss1d(sig):
        g = np.exp(-0.5 * (np.array([-1.0, 0.0, 1.0], np.float32)[None, :] / sig[:, None]) ** 2)
        return g / g.sum(axis=1, keepdims=True)

    gs = gauss1d(sigma_s)  # [N,3]: [aE, aC, aE]
    gf = gauss1d(sigma_f)
    aE, aC = gs[:, 0], gs[:, 1]
    bE = gf[:, 0]

    kpos = max(abs(float(np.asarray(k))), 1.0)
    gt = gain / tau

    pars = np.zeros((NIMG, NPARAM), np.float32)
    pars[:, P_S] = np.sqrt(0.5) / sigma_r
    pars[:, P_LOGE] = np.log(aE * aC)
    pars[:, P_LOGC] = np.log(aE * aE)
    pars[:, P_WSC] = aC * aC
    pars[:, P_BE] = bE
    pars[:, P_KT] = 0.5 * kpos
    pars[:, P_KTB] = -0.5 * kpos * NOISE_THRESH
    pars[:, P_IGT] = 1.0 / gt
    pars[:, P_OFFGT] = offset / gt
    pars[:, P_CLIP] = 10.0 / tau
    pars[:, P_SQL] = np.sqrt(lam * bE / 2.0)
    pars[:, P_NSQL] = -np.sqrt(lam * bE / 2.0)
    pars[:, P_TN] = MEAN_N * SKIP_THRESH / tau
    pars[:, P_NBE] = -bE
    # duplicate per half-image
    pars2 = np.repeat(pars, 2, axis=0)  # [1024, NPARAM]

    in_maps = []
    per_core = HALVES // N_CORES
    for c in range(N_CORES):
        sl = slice(c * per_core, (c + 1) * per_core)
        in_maps.append({
            "xpad": np.ascontiguousarray(halves[sl]),
            "pp": np.ascontiguousarray(pars2[sl]),
        })
    return in_maps


def _host_post(results):
    outs = [r["out"] for r in results]  # each [128, 32, 64, 3]
    full = np.concatenate(outs, axis=0)  # [1024, 32, 64, 3]
    full = full.reshape(NIMG, 2, ROWS_PER_HALF, W, C).reshape(NIMG, H, W, C)
    return full.reshape(B, PP, H, W, C)


# --------------------------------------------------------------------------
# device program
# --------------------------------------------------------------------------

def build_program(cfg=None):
    import concourse.tile as tile
    from concourse import bacc, mybir
    from contextlib import ExitStack

    cfg = cfg or {}
    F32 = mybir.dt.float32
    F16 = mybir.dt.float16 if not cfg.get("fp32_all") else mybir.dt.float32
    ALU = mybir.AluOpType
    AF = mybir.ActivationFunctionType
    repeat = int(cfg.get("repeat", 1))

    nc = bacc.Bacc("TRN2", target_bir_lowering=False, debug=False)
    xdram = nc.dram_tensor("xpad", [128, PR, PC, C], F32, kind="ExternalInput").ap()
    pdram = nc.dram_tensor("pp", [128, NPARAM], F32, kind="ExternalInput").ap()
    odram = nc.dram_tensor("out", [128, ROWS_PER_HALF, W, C], F32, kind="ExternalOutput").ap()

    # taps: name, dr, dc, row slice (slab coords), col slice, bias col
    # W rows are extended to the full slab so the gaussian can reuse its diffs.
    TAPS = [
        ("W", 0, -1, (0, 10), (1, 66), P_LOGE),
        ("NW", -1, -1, (1, 10), (1, 66), P_LOGC),
        ("N", -1, 0, (1, 10), (1, 65), P_LOGE),
        ("NE", -1, 1, (1, 10), (0, 65), P_LOGC),
    ]
    # rows actually needed for the bilateral part of each tap (slab coords)
    BIL_ROWS = {"W": (1, 9), "NW": (1, 10), "N": (1, 10), "NE": (1, 10)}

    with tile.TileContext(nc) as tc:
        with ExitStack() as ctx:
            pool = ctx.enter_context(tc.tile_pool(name="main", bufs=1))

            pp = pool.tile([128, NPARAM], F32, tag="pp", bufs=1)
            nc.sync.dma_start(pp[:], pdram[:])

            def par(col):
                return pp[:, col:col + 1]

            for rep in range(repeat):
              accs = pool.tile([128, 2 * CHUNKS], F32, tag="accs", bufs=1,
                               name=f"accs{rep}")
              o2s = []
              xints = []

              for ch in range(CHUNKS):
                rbase = ch * CR
                slab = pool.tile([128, SLAB_R, PC, C], F32, tag="slab", bufs=2,
                                 name=f"slab{ch}")
                nc.sync.dma_start(slab[:], xdram[:, rbase:rbase + SLAB_R, :, :])
                x_int = slab[:, 1:1 + CR, 1:65, :]
                xints.append(x_int)

                # ---------------- bilateral taps + diff fields ----------------
                nacc = None
                sacc = None
                d16s = {}
                for ti, (name, dr, dc, (rs, re), (cs, ce), bcol) in enumerate(TAPS):
                    nr, ncol = re - rs, ce - cs
                    brs, bre = BIL_ROWS[name]
                    bnr = bre - brs
                    boff = brs - rs  # start of bilateral rows inside the tap tile
                    # f32 diff (exact): diff[q] = x[q + delta] - x[q]
                    diff = pool.tile([128, SLAB_R, PC, C], F32, tag="diff", bufs=1,
                                     name=f"df{name}{ch}")
                    dv = diff[:, 0:nr, 0:ncol, :]
                    nc.vector.tensor_tensor(
                        dv, slab[:, rs + dr:re + dr, cs + dc:ce + dc, :],
                        slab[:, rs:re, cs:ce, :], ALU.subtract)
                    # f16 copy on the GPSIMD engine (keeps DVE free)
                    d16 = pool.tile([128, SLAB_R, PC, C], F16, tag="d16" + name,
                                    bufs=1, name=f"d16{name}{ch}")
                    nc.gpsimd.tensor_copy(d16[:, 0:nr, 0:ncol, :], dv)
                    d16s[name] = d16
                    # bilateral-only rows below
                    bdv = diff[:, boff:boff + bnr, 0:ncol, :]
                    sq = pool.tile([128, SLAB_R - 1, PC, C], F32, tag="sq", bufs=1,
                                   name=f"sq{name}{ch}")
                    sv = sq[:, 0:bnr, 0:ncol, :]
                    nc.scalar.activation(sv, bdv, AF.Square, scale=par(P_S))
                    d2a = pool.tile([128, SLAB_R - 1, PC], F32, tag="d2a", bufs=1,
                                    name=f"d2a{name}{ch}")
                    av = d2a[:, 0:bnr, 0:ncol]
                    nc.vector.tensor_tensor(
                        av, sq[:, 0:bnr, 0:ncol, 0], sq[:, 0:bnr, 0:ncol, 1], ALU.add)
                    d2 = pool.tile([128, SLAB_R - 1, PC], F32, tag="d2", bufs=1,
                                   name=f"d2{name}{ch}")
                    bv = d2[:, 0:bnr, 0:ncol]
                    nc.vector.tensor_tensor(bv, av, sq[:, 0:bnr, 0:ncol, 2], ALU.add)
                    # channel-broadcast exp straight to f16
                    ck3 = pool.tile([128, SLAB_R - 1, PC, C], F16, tag="ck3", bufs=1,
                                    name=f"ck3{name}{ch}")
                    c3v = ck3[:, 0:bnr, 0:ncol, :]
                    nc.scalar.activation(
                        c3v, bv.unsqueeze(3).broadcast_to([128, bnr, ncol, C]),
                        AF.Exp, bias=par(bcol), scale=-1.0)
                    prod = pool.tile([128, SLAB_R - 1, PC, C], F16, tag="prod",
                                     bufs=1, name=f"pr{name}{ch}")
                    pv = prod[:, 0:bnr, 0:ncol, :]
                    nc.vector.tensor_tensor(
                        pv, d16[:, boff:boff + bnr, 0:ncol, :], c3v, ALU.mult)
                    # interior (I) and mirrored (I - delta) windows in tap coords
                    ir0, ic0 = 1 - brs, 1 - cs
                    mr0, mc0 = ir0 - dr, ic0 - dc
                    dif = pool.tile([128, CR, W, C], F16, tag="mac", bufs=3,
                                    name=f"dif{name}{ch}")
                    nc.vector.tensor_tensor(
                        dif[:], prod[:, ir0:ir0 + CR, ic0:ic0 + W, :],
                        prod[:, mr0:mr0 + CR, mc0:mc0 + W, :], ALU.subtract)
                    pair = pool.tile([128, CR, W], F16, tag="macp", bufs=4,
                                     name=f"pair{name}{ch}")
                    if ti == 0:
                        # fold center weight into the first pair sum (DVE STT)
                        nc.vector.scalar_tensor_tensor(
                            pair[:], ck3[:, ir0:ir0 + CR, ic0:ic0 + W, 0], par(P_WSC),
                            ck3[:, mr0:mr0 + CR, mc0:mc0 + W, 0], ALU.add, ALU.add)
                    else:
                        nc.gpsimd.tensor_tensor(
                            pair[:], ck3[:, ir0:ir0 + CR, ic0:ic0 + W, 0],
                            ck3[:, mr0:mr0 + CR, mc0:mc0 + W, 0], ALU.add)
                    if ti == 0:
                        nacc, sacc = dif, pair
                    else:
                        nacc2 = pool.tile([128, CR, W, C], F16, tag="mac", bufs=3,
                                          name=f"nacc{ti}{ch}")
                        nc.vector.tensor_tensor(nacc2[:], nacc[:], dif[:], ALU.add)
                        nacc = nacc2
                        if ti == 3:
                            sacc2 = pool.tile([128, CR, W], F32, tag="macp1", bufs=1,
                                              name=f"s1_{ch}")
                        else:
                            sacc2 = pool.tile([128, CR, W], F16, tag="macp", bufs=4,
                                              name=f"sacc{ti}{ch}")
                        nc.gpsimd.tensor_tensor(sacc2[:], sacc[:], pair[:], ALU.add)
                        sacc = sacc2

                rS = pool.tile([128, CR, W], F32, tag="rS", bufs=1, name=f"rS{ch}")
                nc.vector.reciprocal_approx_fast(rS[:], sacc[:])
                tT = pool.tile([128, CR, W, C], F32, tag="tT32", bufs=1,
                               name=f"tT{ch}")
                nc.vector.tensor_tensor(
                    tT[:], nacc[:],
                    rS[:].unsqueeze(3).broadcast_to([128, CR, W, C]), ALU.mult)
                o1 = pool.tile([128, CR, W, C], F32, tag="o1", bufs=1, name=f"o1_{ch}")
                nc.gpsimd.tensor_tensor(o1[:], tT[:], x_int, ALU.add)

                # ------------- gaussian detail from the diff fields -------------
                # H = dW(+E) - dW, V = dN(+S) - dN, G = H + V,
                # s2 = 2H - H(up) - H(down), detail = bE*(G - bE*s2) = bE*inner
                dW, dN = d16s["W"], d16s["N"]
                Hf = pool.tile([128, SLAB_R, W, C], F32, tag="gau", bufs=4,
                               name=f"H{ch}")
                nc.vector.tensor_tensor(
                    Hf[:], dW[:, 0:SLAB_R, 1:W + 1, :], dW[:, 0:SLAB_R, 0:W, :],
                    ALU.subtract)
                t2g = pool.tile([128, SLAB_R, W, C], F32, tag="gau", bufs=4,
                                name=f"t2g{ch}")
                nc.vector.tensor_tensor(
                    t2g[:, 0:CR, :, :], Hf[:, 1:1 + CR, :, :], dN[:, 1:1 + CR, 0:W, :],
                    ALU.add)
                Gf = pool.tile([128, SLAB_R, W, C], F32, tag="gau", bufs=4,
                               name=f"G{ch}")
                nc.vector.tensor_tensor(
                    Gf[:, 0:CR, :, :], t2g[:, 0:CR, :, :], dN[:, 0:CR, 0:W, :],
                    ALU.subtract)
                s1g = pool.tile([128, SLAB_R, W, C], F32, tag="gau", bufs=4,
                                name=f"s1g{ch}")
                nc.vector.scalar_tensor_tensor(
                    s1g[:, 0:CR, :, :], Hf[:, 1:1 + CR, :, :], 2.0, Hf[:, 0:CR, :, :],
                    ALU.mult, ALU.subtract)
                s2g = pool.tile([128, SLAB_R, W, C], F32, tag="gau", bufs=4,
                                name=f"s2g{ch}")
                nc.vector.tensor_tensor(
                    s2g[:, 0:CR, :, :], s1g[:, 0:CR, :, :], Hf[:, 2:2 + CR, :, :],
                    ALU.subtract)
                inner = pool.tile([128, CR, W, C], F32, tag="inner", bufs=1,
                                  name=f"inner{ch}")
                nc.vector.scalar_tensor_tensor(
                    inner[:], s2g[:, 0:CR, :, :], par(P_NBE), Gf[:, 0:CR, :, :],
                    ALU.mult, ALU.add)

                # ---------------- noise / masks ----------------
                adet = pool.tile([128, CR, W, C], F16, tag="nz16", bufs=4,
                                 name=f"adet{ch}")
                nc.scalar.activation(
                    adet[:], inner[:], AF.Abs, scale=par(P_BE),
                    accum_out=accs[:, ch:ch + 1])
                th = pool.tile([128, CR, W, C], F16, tag="nz16", bufs=4,
                               name=f"th{ch}")
                nc.scalar.activation(
                    th[:], adet[:], AF.Tanh, bias=par(P_KTB), scale=par(P_KT))
                d1 = pool.tile([128, CR, W, C], F32, tag="nz32", bufs=3,
                               name=f"d1_{ch}")
                nc.scalar.activation(
                    d1[:], x_int, AF.Identity, bias=par(P_OFFGT), scale=par(P_IGT))
                r1 = pool.tile([128, CR, W, C], F32, tag="nz32", bufs=3,
                               name=f"r1_{ch}")
                nc.vector.reciprocal_approx_fast(
                    r1[:].rearrange("p a b c -> p (a b c)"),
                    d1[:].rearrange("p a b c -> p (a b c)"))
                ne0 = pool.tile([128, CR, W, C], F32, tag="nz32", bufs=3,
                                name=f"ne0_{ch}")
                nc.vector.tensor_tensor(ne0[:], adet[:], r1[:], ALU.mult)
                neq = pool.tile([128, CR, W, C], F32, tag="nz32", bufs=3,
                                name=f"neq{ch}")
                nc.vector.tensor_scalar(
                    neq[:], ne0[:], par(P_CLIP), None, ALU.min, ALU.add,
                    accum_out=accs[:, CHUNKS + ch:CHUNKS + ch + 1])
                sqn = pool.tile([128, CR, W, C], F32, tag="nz32", bufs=3,
                                name=f"sqn{ch}")
                nc.scalar.activation(sqn[:], neq[:], AF.Square)
                ee = pool.tile([128, CR, W, C], F32, tag="nz32", bufs=3,
                               name=f"ee{ch}")
                nc.scalar.activation(ee[:], sqn[:], AF.Exp, scale=-1.0)
                t2 = pool.tile([128, CR, W, C], F32, tag="nz32", bufs=3,
                               name=f"t2_{ch}")
                nc.vector.tensor_scalar(
                    t2[:], ee[:], par(P_NSQL), par(P_SQL), ALU.mult, ALU.add)
                nm = pool.tile([128, CR, W, C], F16, tag="nz16", bufs=4,
                               name=f"nm{ch}")
                nc.scalar.activation(nm[:], t2[:], AF.Square)
                s3 = pool.tile([128, CR, W, C], F16, tag="nz16", bufs=4,
                               name=f"s3_{ch}")
                nc.vector.scalar_tensor_tensor(
                    s3[:], th[:], 1.0, nm[:], ALU.add, ALU.mult)
                sharp = pool.tile([128, CR, W, C], F32, tag="sharp", bufs=1,
                                  name=f"sharp{ch}")
                nc.vector.tensor_tensor(sharp[:], s3[:], inner[:], ALU.mult)
                o2 = pool.tile([128, CR, W, C], F32, tag="o2", bufs=CHUNKS,
                               name=f"o2_{ch}")
                nc.gpsimd.tensor_tensor(o2[:], o1[:], sharp[:], ALU.add)
                o2s.append(o2)

              # ---------------- skip flags ----------------
              my2 = pool.tile([128, 2], F32, tag="fl", bufs=1, name=f"my2{rep}")
              t_a = pool.tile([128, 2], F32, tag="fl2", bufs=1, name=f"ta{rep}")
              nc.vector.tensor_tensor(t_a[:], accs[:, 0:2], accs[:, 2:4], ALU.add)
              nc.vector.tensor_tensor(my2[:, 0:1], t_a[:, 0:1], t_a[:, 1:2], ALU.add)
              t_n = pool.tile([128, 2], F32, tag="fl3", bufs=1, name=f"tn{rep}")
              nc.vector.tensor_tensor(
                  t_n[:], accs[:, CHUNKS:CHUNKS + 2], accs[:, CHUNKS + 2:CHUNKS + 4],
                  ALU.add)
              nc.vector.tensor_tensor(my2[:, 1:2], t_n[:, 0:1], t_n[:, 1:2], ALU.add)
              other2 = pool.tile([128, 2], F32, tag="fl4", bufs=1, name=f"oth{rep}")
              nc.vector.stream_shuffle(other2[:], my2[:], [i ^ 1 for i in range(32)])
              tot = pool.tile([128, 2], F32, tag="fl5", bufs=1, name=f"tot{rep}")
              nc.vector.tensor_tensor(tot[:], my2[:], other2[:], ALU.add)
              fa = pool.tile([128, 1], F32, tag="fl6", bufs=1, name=f"fa{rep}")
              nc.vector.tensor_scalar(
                  fa[:], tot[:, 0:1], MEAN_N * SKIP_THRESH, None, ALU.is_lt)
              fn = pool.tile([128, 1], F32, tag="fl7", bufs=1, name=f"fn{rep}")
              nc.vector.tensor_scalar(fn[:], tot[:, 1:2], par(P_TN), None, ALU.is_lt)
              fl = pool.tile([128, 1], F32, tag="fl8", bufs=1, name=f"fl{rep}")
              nc.vector.tensor_tensor(fl[:], fa[:], fn[:], ALU.max)
              w1 = pool.tile([128, 1], F32, tag="fl9", bufs=1, name=f"w1{rep}")
              nc.vector.tensor_scalar(w1[:], fl[:], -1.0, 1.0, ALU.mult, ALU.add)

              # ---------------- blend + clip + store ----------------
              for ch in range(CHUNKS):
                # slabs are long recycled by now -> re-read the interior rows
                xre = pool.tile([128, CR, W, C], F32, tag="xre", bufs=1,
                                name=f"xre{ch}_{rep}")
                nc.sync.dma_start(
                    xre[:], xdram[:, ch * CR + 1:ch * CR + 1 + CR, 1:65, :])
                xw = pool.tile([128, CR, W, C], F32, tag="xw", bufs=1,
                               name=f"xw{ch}_{rep}")
                nc.scalar.activation(xw[:], xre[:], AF.Copy, scale=fl[:])
                o2b = pool.tile([128, CR, W, C], F32, tag="o2b", bufs=1,
                                name=f"o2b{ch}_{rep}")
                nc.vector.scalar_tensor_tensor(
                    o2b[:], o2s[ch][:], w1[:], xw[:], ALU.mult, ALU.add)
                o3 = pool.tile([128, CR, W, C], F32, tag="o3", bufs=1,
                               name=f"o3_{ch}_{rep}")
                nc.vector.tensor_scalar(o3[:], o2b[:], 1e-5, 1.0, ALU.max, ALU.min)
                nc.sync.dma_start(odram[:, ch * CR:(ch + 1) * CR, :, :], o3[:])

    nc.compile()
    return nc


def _get_program(cfg=None):
    key = tuple(sorted((cfg or {}).items()))
    if key not in _CACHE:
        _CACHE[key] = build_program(cfg)
    return _CACHE[key]


# --------------------------------------------------------------------------
# entry point
# --------------------------------------------------------------------------

def kernel(images, params, k):
    from concourse.bass_utils import run_bass_kernel_spmd

    nc = _get_program()
    in_maps = _host_prep(np.asarray(images), np.asarray(params), np.asarray(k))
    res = run_bass_kernel_spmd(nc, in_maps, list(range(N_CORES)))
    return _host_post(res.results).astype(np.float32)


# revision 14
# speedup vs baseline: 470.8000x; 470.8000x over previous
"""Trainium2 Bass kernel for nn_DenoisingSharpening.

Contract: kernel(**inputs) takes the FULL unsharded inputs
(images [8,64,64,64,3] f32, params [8,64,7] f32, k [] f32) and returns
the FULL output [8,64,64,64,3] f32.

Strategy
--------
Embarrassingly data-parallel over the N = B*P = 512 images; 64 images per
NeuronCore, one half-image (32 rows) per SBUF partition -> 128 partitions.

Host side: reflect-pads each image to [34, 66, 3] halves and precomputes all
per-image scalars (gaussian kernels, bilateral scales, thresholds) so the
device only streams elementwise math.

Device side (per core, Tile framework):
  * bilateral 3x3: only 4 unique tap fields (W, NW, N, NE) are computed;
    the mirror taps are shifted reads of the same fields (the color kernel
    is symmetric between pixel pairs).
  * neighbor diffs are computed in f32 (exact), then converted to f16 on the
    GPSIMD engine; products/accumulations run as f16 tensor_tensor (2x DVE).
  * ck3 = exp(-d2 + log(ws)) is emitted channel-broadcast in f16 directly by
    the ACT engine (free scale/bias + broadcast read).
  * bf = x + sum(ck*diff)/sum(ck) (algebraically identical to the reference).
  * the separable gaussian detail is reconstructed from the SAME diff fields
    (diff-space formulation) so f16 stays precision-safe:
        H = dW + dW(+E), V = dN + dN(+S), G = H + V,
        s2 = 2H - H(up) - H(down),
        detail = -bE*(G - bE*s2) = -bE*inner
    with the -bE folded into abs-scale and the lambda scalar.
  * detail-mask via tanh (same ACT table set as exp/square/abs -> one
    table load), noise chain fused via tensor_scalar / STT.
  * skip-means via free accum_out reductions + stream_shuffle partition
    pair exchange.
"""

import numpy as np

N_CORES = 8
B, PP, H, W, C = 8, 64, 64, 64, 3
NIMG = B * PP  # 512
HALVES = 2 * NIMG  # 1024 half-images, 128 per core
PR, PC = 34, 66  # padded half-image rows/cols
ROWS_PER_HALF = 32
CHUNKS = 4
CR = ROWS_PER_HALF // CHUNKS  # interior rows per chunk (8)
SLAB_R = CR + 2  # slab rows incl. halo (10)

NOISE_THRESH = 0.002
SKIP_THRESH = 1e-4
MEAN_N = float(C * H * W)  # 12288 elements per image mean

# params columns
(P_S, P_LOGE, P_LOGC, P_WSC, P_BE, P_KT, P_KTB, P_IGT, P_OFFGT, P_CLIP,
 P_SQL, P_NSQL, P_TN, P_NBE, P_PAD1, P_PAD2) = range(16)
NPARAM = 16

_CACHE = {}


# --------------------------------------------------------------------------
# host-side preprocessing
# --------------------------------------------------------------------------

def _host_prep(images, params, k):
    x = np.ascontiguousarray(images, dtype=np.float32).reshape(NIMG, H, W, C)
    xp = np.pad(x, ((0, 0), (1, 1), (1, 1), (0, 0)), mode="reflect")
    # halves: rows 0..33 and 32..65 of the padded [66,66,3]
    halves = np.stack([xp[:, 0:PR], xp[:, ROWS_PER_HALF:ROWS_PER_HALF + PR]], axis=1)
    halves = np.ascontiguousarray(halves, dtype=np.float32).reshape(HALVES, PR, PC, C)

    p = np.asarray(params, dtype=np.float32).reshape(NIMG, 7)
    sigma_s = np.clip(p[:, 0], 0.2, 5.0)
    sigma_r = np.clip(p[:, 1], 0.01, 1.0)
    sigma_f = np.clip(p[:, 2], 0.2, 3.0)
    lam = np.clip(p[:, 3], 0.1, 2.0)
    tau = np.clip(p[:, 4], 0.5, 5.0)
    gain = np.clip(p[:, 5], 0.2, 2.0)
    offset = np.clip(p[:, 6], 0.01, 1.0)

    def gauss1d(sig):
        g = np.exp(-0.5 * (np.array([-1.0, 0.0, 1.0], np.float32)[None, :] / sig[:, None]) ** 2)
        return g / g.sum(axis=1, keepdims=True)

    gs = gauss1d(sigma_s)  # [N,3]: [aE, aC, aE]
    gf = gauss1d(sigma_f)
    aE, aC = gs[:, 0], gs[:, 1]
    bE = gf[:, 0]

    kpos = max(abs(float(np.asarray(k))), 1.0)
    gt = gain / tau

    pars = np.zeros((NIMG, NPARAM), np.float32)
    pars[:, P_S] = np.sqrt(0.5) / sigma_r
    pars[:, P_LOGE] = np.log(aE * aC)
    pars[:, P_LOGC] = np.log(aE * aE)
    pars[:, P_WSC] = aC * aC
    pars[:, P_BE] = bE
    pars[:, P_KT] = 0.5 * kpos
    pars[:, P_KTB] = -0.5 * kpos * NOISE_THRESH
    pars[:, P_IGT] = 1.0 / gt
    pars[:, P_OFFGT] = offset / gt
    pars[:, P_CLIP] = 10.0 / tau
    pars[:, P_SQL] = np.sqrt(lam * bE / 2.0)
    pars[:, P_NSQL] = -np.sqrt(lam * bE / 2.0)
    pars[:, P_TN] = MEAN_N * SKIP_THRESH / tau
    pars[:, P_NBE] = -bE
    # duplicate per half-image
    pars2 = np.repeat(pars, 2, axis=0)  # [1024, NPARAM]

    in_maps = []
    per_core = HALVES // N_CORES
    for c in range(N_CORES):
        sl = slice(c * per_core, (c + 1) * per_core)
        in_maps.append({
            "xpad": np.ascontiguousarray(halves[sl]),
            "pp": np.ascontiguousarray(pars2[sl]),
        })
    return in_maps


def _host_post(results):
    outs = [r["out"] for r in results]  # each [128, 32, 64, 3]
    full = np.concatenate(outs, axis=0)  # [1024, 32, 64, 3]
    full = full.reshape(NIMG, 2, ROWS_PER_HALF, W, C).reshape(NIMG, H, W, C)
    return full.reshape(B, PP, H, W, C)


# --------------------------------------------------------------------------
# device program
# --------------------------------------------------------------------------

def build_program(cfg=None):
    import concourse.tile as tile
    from concourse import bacc, mybir
    from contextlib import ExitStack

    cfg = cfg or {}
    F32 = mybir.dt.float32
    F16 = mybir.dt.float16 if not cfg.get("fp32_all") else mybir.dt.float32
    ALU = mybir.AluOpType
    AF = mybir.ActivationFunctionType
    repeat = int(cfg.get("repeat", 1))
    mac32 = bool(cfg.get("mac32", False))
    FMAC = F32 if mac32 else F16

    nc = bacc.Bacc("TRN2", target_bir_lowering=False, debug=False)
    xdram = nc.dram_tensor("xpad", [128, PR, PC, C], F32, kind="ExternalInput").ap()
    pdram = nc.dram_tensor("pp", [128, NPARAM], F32, kind="ExternalInput").ap()
    odram = nc.dram_tensor("out", [128, ROWS_PER_HALF, W, C], F32, kind="ExternalOutput").ap()

    # taps: name, dr, dc, row slice (slab coords), col slice, bias col
    # W rows are extended to the full slab so the gaussian can reuse its diffs.
    TAPS = [
        ("W", 0, -1, (0, 10), (1, 66), P_LOGE),
        ("NW", -1, -1, (1, 10), (1, 66), P_LOGC),
        ("N", -1, 0, (1, 10), (1, 65), P_LOGE),
        ("NE", -1, 1, (1, 10), (0, 65), P_LOGC),
    ]
    if mac32:
        TAPS = [TAPS[1], TAPS[3], TAPS[0], TAPS[2]]
    # rows actually needed for the bilateral part of each tap (slab coords)
    BIL_ROWS = {"W": (1, 9), "NW": (1, 10), "N": (1, 10), "NE": (1, 10)}

    with tile.TileContext(nc) as tc:
        with ExitStack() as ctx:
            pool = ctx.enter_context(tc.tile_pool(name="main", bufs=1))

            pp = pool.tile([128, NPARAM], F32, tag="pp", bufs=1)
            nc.sync.dma_start(pp[:], pdram[:])

            def par(col):
                return pp[:, col:col + 1]

            for rep in range(repeat):
              accs = pool.tile([128, 2 * CHUNKS], F32, tag="accs", bufs=1,
                               name=f"accs{rep}")
              o2s = []
              xints = []

              for ch in range(CHUNKS):
                rbase = ch * CR
                slab = pool.tile([128, SLAB_R, PC, C], F32, tag="slab", bufs=1 if mac32 else 2,
                                 name=f"slab{ch}")
                nc.sync.dma_start(slab[:], xdram[:, rbase:rbase + SLAB_R, :, :])
                x_int = slab[:, 1:1 + CR, 1:65, :]
                xints.append(x_int)

                # ---------------- bilateral taps + diff fields ----------------
                nacc = None
                sacc = None
                d16s = {}
                for ti, (name, dr, dc, (rs, re), (cs, ce), bcol) in enumerate(TAPS):
                    nr, ncol = re - rs, ce - cs
                    brs, bre = BIL_ROWS[name]
                    bnr = bre - brs
                    boff = brs - rs  # start of bilateral rows inside the tap tile
                    # f32 diff (exact): diff[q] = x[q + delta] - x[q]
                    diff = pool.tile([128, SLAB_R, PC, C], F32, tag="diff",
                                     bufs=2 if mac32 else 1,
                                     name=f"df{name}{ch}")
                    dv = diff[:, 0:nr, 0:ncol, :]
                    nc.vector.tensor_tensor(
                        dv, slab[:, rs + dr:re + dr, cs + dc:ce + dc, :],
                        slab[:, rs:re, cs:ce, :], ALU.subtract)
                    # f16 copy on the GPSIMD engine (keeps DVE free)
                    if mac32:
                        d16s[name] = diff
                    else:
                        d16 = pool.tile([128, SLAB_R, PC, C], F16, tag="d16" + name,
                                        bufs=1, name=f"d16{name}{ch}")
                        nc.gpsimd.tensor_copy(d16[:, 0:nr, 0:ncol, :], dv)
                        d16s[name] = d16
                    # bilateral-only rows below
                    bdv = diff[:, boff:boff + bnr, 0:ncol, :]
                    sq = pool.tile([128, SLAB_R - 1, PC, C], F32, tag="sq", bufs=1,
                                   name=f"sq{name}{ch}")
                    sv = sq[:, 0:bnr, 0:ncol, :]
                    nc.scalar.activation(sv, bdv, AF.Square, scale=par(P_S))
                    d2a = pool.tile([128, SLAB_R - 1, PC], F32, tag="d2a", bufs=1,
                                    name=f"d2a{name}{ch}")
                    av = d2a[:, 0:bnr, 0:ncol]
                    nc.vector.tensor_tensor(
                        av, sq[:, 0:bnr, 0:ncol, 0], sq[:, 0:bnr, 0:ncol, 1], ALU.add)
                    d2 = pool.tile([128, SLAB_R - 1, PC], F32, tag="d2", bufs=1,
                                   name=f"d2{name}{ch}")
                    bv = d2[:, 0:bnr, 0:ncol]
                    nc.vector.tensor_tensor(bv, av, sq[:, 0:bnr, 0:ncol, 2], ALU.add)
                    # channel-broadcast exp straight to f16
                    ck3 = pool.tile([128, SLAB_R - 1, PC, C], FMAC, tag="ck3", bufs=1,
                                    name=f"ck3{name}{ch}")
                    c3v = ck3[:, 0:bnr, 0:ncol, :]
                    nc.scalar.activation(
                        c3v, bv.unsqueeze(3).broadcast_to([128, bnr, ncol, C]),
                        AF.Exp, bias=par(bcol), scale=-1.0)
                    prod = pool.tile([128, SLAB_R - 1, PC, C], FMAC, tag="prod",
                                     bufs=1, name=f"pr{name}{ch}")
                    pv = prod[:, 0:bnr, 0:ncol, :]
                    nc.vector.tensor_tensor(
                        pv, d16s[name][:, boff:boff + bnr, 0:ncol, :], c3v, ALU.mult)
                    # interior (I) and mirrored (I - delta) windows in tap coords
                    ir0, ic0 = 1 - brs, 1 - cs
                    mr0, mc0 = ir0 - dr, ic0 - dc
                    dif = pool.tile([128, CR, W, C], FMAC, tag="mac", bufs=3,
                                    name=f"dif{name}{ch}")
                    nc.vector.tensor_tensor(
                        dif[:], prod[:, ir0:ir0 + CR, ic0:ic0 + W, :],
                        prod[:, mr0:mr0 + CR, mc0:mc0 + W, :], ALU.subtract)
                    pair = pool.tile([128, CR, W], FMAC, tag="macp", bufs=4,
                                     name=f"pair{name}{ch}")
                    if ti == 0:
                        # fold center weight into the first pair sum (DVE STT)
                        nc.vector.scalar_tensor_tensor(
                            pair[:], ck3[:, ir0:ir0 + CR, ic0:ic0 + W, 0], par(P_WSC),
                            ck3[:, mr0:mr0 + CR, mc0:mc0 + W, 0], ALU.add, ALU.add)
                    else:
                        nc.gpsimd.tensor_tensor(
                            pair[:], ck3[:, ir0:ir0 + CR, ic0:ic0 + W, 0],
                            ck3[:, mr0:mr0 + CR, mc0:mc0 + W, 0], ALU.add)
                    if ti == 0:
                        nacc, sacc = dif, pair
                    else:
                        nacc2 = pool.tile([128, CR, W, C], FMAC, tag="mac", bufs=3,
                                          name=f"nacc{ti}{ch}")
                        nc.vector.tensor_tensor(nacc2[:], nacc[:], dif[:], ALU.add)
                        nacc = nacc2
                        if ti == 3:
                            sacc2 = pool.tile([128, CR, W], F32, tag="macp1", bufs=1,
                                              name=f"s1_{ch}")
                        else:
                            sacc2 = pool.tile([128, CR, W], FMAC, tag="macp", bufs=4,
                                              name=f"sacc{ti}{ch}")
                        nc.gpsimd.tensor_tensor(sacc2[:], sacc[:], pair[:], ALU.add)
                        sacc = sacc2

                rS = pool.tile([128, CR, W], F32, tag="rS", bufs=1, name=f"rS{ch}")
                nc.vector.reciprocal_approx_fast(rS[:], sacc[:])
                tT = pool.tile([128, CR, W, C], F32, tag="tT32", bufs=1,
                               name=f"tT{ch}")
                nc.vector.tensor_tensor(
                    tT[:], nacc[:],
                    rS[:].unsqueeze(3).broadcast_to([128, CR, W, C]), ALU.mult)
                o1 = pool.tile([128, CR, W, C], F32, tag="o1", bufs=1, name=f"o1_{ch}")
                nc.gpsimd.tensor_tensor(o1[:], tT[:], x_int, ALU.add)

                # ------------- gaussian detail from the diff fields -------------
                # H = dW(+E) - dW, V = dN(+S) - dN, G = H + V,
                # s2 = 2H - H(up) - H(down), detail = bE*(G - bE*s2) = bE*inner
                dW, dN = d16s["W"], d16s["N"]
                Hf = pool.tile([128, SLAB_R, W, C], F32, tag="gau", bufs=4,
                               name=f"H{ch}")
                nc.vector.tensor_tensor(
                    Hf[:], dW[:, 0:SLAB_R, 1:W + 1, :], dW[:, 0:SLAB_R, 0:W, :],
                    ALU.subtract)
                t2g = pool.tile([128, SLAB_R, W, C], F32, tag="gau", bufs=4,
                                name=f"t2g{ch}")
                nc.vector.tensor_tensor(
                    t2g[:, 0:CR, :, :], Hf[:, 1:1 + CR, :, :], dN[:, 1:1 + CR, 0:W, :],
                    ALU.add)
                Gf = pool.tile([128, SLAB_R, W, C], F32, tag="gau", bufs=4,
                               name=f"G{ch}")
                nc.vector.tensor_tensor(
                    Gf[:, 0:CR, :, :], t2g[:, 0:CR, :, :], dN[:, 0:CR, 0:W, :],
                    ALU.subtract)
                s1g = pool.tile([128, SLAB_R, W, C], F32, tag="gau", bufs=4,
                                name=f"s1g{ch}")
                nc.vector.scalar_tensor_tensor(
                    s1g[:, 0:CR, :, :], Hf[:, 1:1 + CR, :, :], 2.0, Hf[:, 0:CR, :, :],
                    ALU.mult, ALU.subtract)
                s2g = pool.tile([128, SLAB_R, W, C], F32, tag="gau", bufs=4,
                                name=f"s2g{ch}")
                nc.vector.tensor_tensor(
                    s2g[:, 0:CR, :, :], s1g[:, 0:CR, :, :], Hf[:, 2:2 + CR, :, :],
                    ALU.subtract)
                inner = pool.tile([128, CR, W, C], F32, tag="inner", bufs=1,
                                  name=f"inner{ch}")
                nc.vector.scalar_tensor_tensor(
                    inner[:], s2g[:, 0:CR, :, :], par(P_NBE), Gf[:, 0:CR, :, :],
                    ALU.mult, ALU.add)

                # ---------------- noise / masks ----------------
                adet = pool.tile([128, CR, W, C], F16, tag="nz16", bufs=3 if mac32 else 4,
                                 name=f"adet{ch}")
                nc.scalar.activation(
                    adet[:], inner[:], AF.Abs, scale=par(P_BE),
                    accum_out=accs[:, ch:ch + 1])
                th = pool.tile([128, CR, W, C], F16, tag="nz16", bufs=3 if mac32 else 4,
                               name=f"th{ch}")
                nc.scalar.activation(
                    th[:], adet[:], AF.Tanh, bias=par(P_KTB), scale=par(P_KT))
                d1 = pool.tile([128, CR, W, C], F32, tag="nz32", bufs=2 if mac32 else 3,
                               name=f"d1_{ch}")
                nc.scalar.activation(
                    d1[:], x_int, AF.Identity, bias=par(P_OFFGT), scale=par(P_IGT))
                r1 = pool.tile([128, CR, W, C], F32, tag="nz32", bufs=2 if mac32 else 3,
                               name=f"r1_{ch}")
                nc.vector.reciprocal_approx_fast(
                    r1[:].rearrange("p a b c -> p (a b c)"),
                    d1[:].rearrange("p a b c -> p (a b c)"))
                ne0 = pool.tile([128, CR, W, C], F32, tag="nz32", bufs=2 if mac32 else 3,
                                name=f"ne0_{ch}")
                nc.vector.tensor_tensor(ne0[:], adet[:], r1[:], ALU.mult)
                neq = pool.tile([128, CR, W, C], F32, tag="nz32", bufs=2 if mac32 else 3,
                                name=f"neq{ch}")
                nc.vector.tensor_scalar(
                    neq[:], ne0[:], par(P_CLIP), None, ALU.min, ALU.add,
                    accum_out=accs[:, CHUNKS + ch:CHUNKS + ch + 1])
                sqn = pool.tile([128, CR, W, C], F32, tag="nz32", bufs=2 if mac32 else 3,
                                name=f"sqn{ch}")
                nc.scalar.activation(sqn[:], neq[:], AF.Square)
                ee = pool.tile([128, CR, W, C], F32, tag="nz32", bufs=2 if mac32 else 3,
                               name=f"ee{ch}")
                nc.scalar.activation(ee[:], sqn[:], AF.Exp, scale=-1.0)
                t2 = pool.tile([128, CR, W, C], F32, tag="nz32", bufs=2 if mac32 else 3,
                               name=f"t2_{ch}")
                nc.vector.tensor_scalar(
                    t2[:], ee[:], par(P_NSQL), par(P_SQL), ALU.mult, ALU.add)
                nm = pool.tile([128, CR, W, C], F16, tag="nz16", bufs=3 if mac32 else 4,
                               name=f"nm{ch}")
                nc.scalar.activation(nm[:], t2[:], AF.Square)
                s3 = pool.tile([128, CR, W, C], F16, tag="nz16", bufs=3 if mac32 else 4,
                               name=f"s3_{ch}")
                nc.vector.scalar_tensor_tensor(
                    s3[:], th[:], 1.0, nm[:], ALU.add, ALU.mult)
                sharp = pool.tile([128, CR, W, C], F32, tag="sharp", bufs=1,
                                  name=f"sharp{ch}")
                nc.vector.tensor_tensor(sharp[:], s3[:], inner[:], ALU.mult)
                o2 = pool.tile([128, CR, W, C], F32, tag="o2", bufs=CHUNKS,
                               name=f"o2_{ch}")
                nc.gpsimd.tensor_tensor(o2[:], o1[:], sharp[:], ALU.add)
                o2s.append(o2)

              # ---------------- skip flags ----------------
              my2 = pool.tile([128, 2], F32, tag="fl", bufs=1, name=f"my2{rep}")
              t_a = pool.tile([128, 2], F32, tag="fl2", bufs=1, name=f"ta{rep}")
              nc.vector.tensor_tensor(t_a[:], accs[:, 0:2], accs[:, 2:4], ALU.add)
              nc.vector.tensor_tensor(my2[:, 0:1], t_a[:, 0:1], t_a[:, 1:2], ALU.add)
              t_n = pool.tile([128, 2], F32, tag="fl3", bufs=1, name=f"tn{rep}")
              nc.vector.tensor_tensor(
                  t_n[:], accs[:, CHUNKS:CHUNKS + 2], accs[:, CHUNKS + 2:CHUNKS + 4],
                  ALU.add)
              nc.vector.tensor_tensor(my2[:, 1:2], t_n[:, 0:1], t_n[:, 1:2], ALU.add)
              other2 = pool.tile([128, 2], F32, tag="fl4", bufs=1, name=f"oth{rep}")
              nc.vector.stream_shuffle(other2[:], my2[:], [i ^ 1 for i in range(32)])
              tot = pool.tile([128, 2], F32, tag="fl5", bufs=1, name=f"tot{rep}")
              nc.vector.tensor_tensor(tot[:], my2[:], other2[:], ALU.add)
              fa = pool.tile([128, 1], F32, tag="fl6", bufs=1, name=f"fa{rep}")
              nc.vector.tensor_scalar(
                  fa[:], tot[:, 0:1], MEAN_N * SKIP_THRESH, None, ALU.is_lt)
              fn = pool.tile([128, 1], F32, tag="fl7", bufs=1, name=f"fn{rep}")
              nc.vector.tensor_scalar(fn[:], tot[:, 1:2], par(P_TN), None, ALU.is_lt)
              fl = pool.tile([128, 1], F32, tag="fl8", bufs=1, name=f"fl{rep}")
              nc.vector.tensor_tensor(fl[:], fa[:], fn[:], ALU.max)
              w1 = pool.tile([128, 1], F32, tag="fl9", bufs=1, name=f"w1{rep}")
              nc.vector.tensor_scalar(w1[:], fl[:], -1.0, 1.0, ALU.mult, ALU.add)

              # ---------------- blend + clip + store ----------------
              for ch in range(CHUNKS):
                # slabs are long recycled by now -> re-read the interior rows
                xre = pool.tile([128, CR, W, C], F32, tag="xre", bufs=1,
                                name=f"xre{ch}_{rep}")
                nc.sync.dma_start(
                    xre[:], xdram[:, ch * CR + 1:ch * CR + 1 + CR, 1:65, :])
                xw = pool.tile([128, CR, W, C], F32, tag="xw", bufs=1,
                               name=f"xw{ch}_{rep}")
                nc.scalar.activation(xw[:], xre[:], AF.Copy, scale=fl[:])
                o2b = pool.tile([128, CR, W, C], F32, tag="o2b", bufs=1,
                                name=f"o2b{ch}_{rep}")
                nc.vector.scalar_tensor_tensor(
                    o2b[:], o2s[ch][:], w1[:], xw[:], ALU.mult, ALU.add)
                o3 = pool.tile([128, CR, W, C], F32, tag="o3", bufs=1,
                               name=f"o3_{ch}_{rep}")
                nc.vector.tensor_scalar(o3[:], o2b[:], 1e-5, 1.0, ALU.max, ALU.min)
                nc.sync.dma_start(odram[:, ch * CR:(ch + 1) * CR, :, :], o3[:])

    nc.compile()
    return nc


def _get_program(cfg=None):
    key = tuple(sorted((cfg or {}).items()))
    if key not in _CACHE:
        _CACHE[key] = build_program(cfg)
    return _CACHE[key]


# --------------------------------------------------------------------------
# entry point
# --------------------------------------------------------------------------

def kernel(images, params, k):
    from concourse.bass_utils import run_bass_kernel_spmd

    nc = _get_program({"mac32": True})
    in_maps = _host_prep(np.asarray(images), np.asarray(params), np.asarray(k))
    res = run_bass_kernel_spmd(nc, in_maps, list(range(N_CORES)))
    return _host_post(res.results).astype(np.float32)
